# revision 19
# baseline (speedup 1.0000x reference)
"""Trainium2 Bass/Tile kernel for BasicCondConvBlock (E=1):
two CondConv1d(k=3,pad=1)+BN(eval)+LeakyReLU(0.1) blocks + MaxPool1d(2).

With a single expert, CondConv reduces to y_i = r_i * (conv(x_i, W) + b)
with a shared weight: conv runs on the TensorEngine as 3 shifted fp32r
matmuls accumulated in PSUM, and routing r_i + conv bias + BatchNorm fold
into one per-(sample,channel) affine.  The whole block-1 epilogue is a
single ScalarE op per tile: Prelu(z*scale + bias, alpha=0.1) (= LeakyReLU)
writing fp32r with a fused row-sum (feeds block-2 routing).  Block 2 pools
first — max over adjacent pairs straight out of PSUM via a one-input
3D-AP tensor_reduce (exact since scale>0 keeps affine+Prelu monotone) —
then applies Prelu(affine) at half width.

DMA layout: weights/consts ride the ScalarE HWDGE ring while the bulk x
input and output tiles go through SWDGE (gpsimd) — measured ~3x the HWDGE
ring's throughput for 1 MiB transfers — so both streams flow concurrently;
routing for the first sample pair is computed from the first x tile only,
keeping the PE busy (and its HAM clock warm) from the first conv matmul on.

Sharding: pure data parallel over batch (32 samples -> 4 per core x 8).
"""

import numpy as np

N_CORES = 8
B, CIN, W = 32, 64, 2048
C1, C2 = 128, 256
BL = B // N_CORES  # samples per core
EPS = 1e-5
SLOPE = 0.1
WT = 512           # conv output tile width (one PSUM bank of fp32)
NT = W // WT       # 4
WO = W // 2        # pooled output width
HT = WT // 2

# packed parameter-row offsets
OFF_S1, OFF_T11, OFF_T21 = 0, C1, 2 * C1
OFF_S2, OFF_T12, OFF_T22 = 3 * C1, 3 * C1 + C2, 3 * C1 + 2 * C2
OFF_FCB1 = 3 * C1 + 3 * C2
OFF_FCB2 = OFF_FCB1 + 1
OFF_ONES = OFF_FCB2 + 1
NPROW = OFF_ONES + BL

TRACE = False
LAST_RESULT = None

_built = None


def _build():
    global _built
    if _built is not None:
        return _built

    import concourse.bacc as bacc
    import concourse.mybir as mybir
    from concourse import tile
    from contextlib import ExitStack

    f32 = mybir.dt.float32
    f32r = mybir.dt.float32r
    Alu = mybir.AluOpType
    Act = mybir.ActivationFunctionType
    Ax = mybir.AxisListType

    nc = bacc.Bacc("TRN2", target_bir_lowering=False, debug=False)

    xd = nc.declare_dram_parameter("x", [BL, CIN, W + 2], f32r, isOutput=False)
    w1d = nc.declare_dram_parameter("wpk1", [2 * CIN, 3 * C1 + 1], f32r, isOutput=False)
    w2d = nc.declare_dram_parameter("wpk2", [C1, 3 * C2 + 1], f32r, isOutput=False)
    pd = nc.declare_dram_parameter("prow", [1, NPROW], f32, isOutput=False)
    od = nc.declare_dram_parameter("out", [BL, C2, WO], f32, isOutput=True)
    x_ap, w1_ap, w2_ap, p_ap, o_ap = xd.ap(), w1d.ap(), w2d.ap(), pd.ap(), od.ap()

    def conv_taps(zp, lhsT, src, c0):
        """Accumulate the 3-tap conv for output cols [c0, c0+WT) of one
        128-wide output-channel chunk.  lhsT(k) -> [K,128] stationary AP;
        src -> [K, W+2] zero-padded input AP (data at cols 1..W).  All taps
        full width: fp32r matmuls need even N and 8B-aligned PSUM offsets."""
        for k in range(3):
            nc.tensor.matmul(zp[:, 0:WT], lhsT(k), src[:, c0 + k : c0 + k + WT],
                             start=(k == 0), stop=(k == 2))

    with tile.TileContext(nc) as tc:
        with ExitStack() as ctx:
            consts = ctx.enter_context(tc.tile_pool(name="consts", bufs=1))
            xpool = ctx.enter_context(tc.tile_pool(name="xp", bufs=2))
            y1pool = ctx.enter_context(tc.tile_pool(name="y1p", bufs=BL))
            pmp = ctx.enter_context(tc.tile_pool(name="pmp", bufs=8))
            outp = ctx.enter_context(tc.tile_pool(name="outp", bufs=3))
            small = ctx.enter_context(tc.tile_pool(name="small", bufs=1))
            psum = ctx.enter_context(tc.tile_pool(name="psum", bufs=7, space="PSUM"))
            psmall = ctx.enter_context(tc.tile_pool(name="psm", bufs=1, space="PSUM"))

            # --- input DMAs: bulk data on SWDGE (gpsimd; ~3x the HWDGE ring
            # throughput at these sizes), w1 weights first, then x one sample
            # at a time so the first conv starts after ~0.5 MiB has landed.
            # Small consts ride the ScalarE HWDGE ring concurrently.
            w1s = consts.tile([2 * CIN, 3 * C1 + 1], f32r)
            nc.gpsimd.dma_start(out=w1s[:], in_=w1_ap[:])
            prs = consts.tile([1, NPROW], f32)
            nc.scalar.dma_start(out=prs[:], in_=p_ap[:])
            w2s = consts.tile([C1, 3 * C2 + 1], f32r)
            nc.scalar.dma_start(out=w2s[:], in_=w2_ap[:])

            xts = [
                xpool.tile([2 * CIN, W + 2], f32r, tag="xt", name=f"xt{i}")
                for i in range(BL // 2)
            ]
            for s in range(BL):
                nc.gpsimd.dma_start(
                    out=xts[s // 2][(s % 2) * CIN : (s % 2 + 1) * CIN, :],
                    in_=x_ap[s],
                )

            def xv(s):
                return xts[s // 2][(s % 2) * CIN : (s % 2 + 1) * CIN, :]

            # per-pair column sums of x -> routing 1 (each scan covers two
            # samples at once on partitions 0:64 / 64:128).  Pair 0 on DVE,
            # pair 1 on ScalarE (copy w/ fused accumulate) so both finish
            # right after their tile lands.
            m1 = small.tile([2 * CIN, BL // 2], f32)
            nc.vector.reduce_sum(
                m1[:, 0:1], xts[0][:, 1 : W + 1].bitcast(f32), axis=Ax.X
            )
            sct = small.tile([2 * CIN, W], f32)
            nc.scalar.activation(
                sct[:], xts[1][:, 1 : W + 1].bitcast(f32), Act.Copy,
                accum_out=m1[:, 1:2],
            )

            ones = prs[0:1, OFF_ONES : OFF_ONES + BL]

            def outer_pair(s_off, t1_off, t2_off, r_row, ncols, sc, bi, cc0):
                """sc[:, cc0:cc0+ncols] = s_c*r_i ; bi[...] = t1_c*r_i + t2_c"""
                cw = C1
                opa = psmall.tile([cw, ncols], f32, tag="sm", name=f"opa{s_off}_{cc0}")
                nc.tensor.matmul(
                    opa[:], prs[0:1, s_off : s_off + cw], r_row, start=True, stop=True
                )
                nc.scalar.activation(sc[:, cc0 : cc0 + ncols], opa[:], Act.Copy)
                opb = psmall.tile([cw, ncols], f32, tag="sm", name=f"opb{t1_off}_{cc0}")
                nc.tensor.matmul(
                    opb[:], prs[0:1, t1_off : t1_off + cw], r_row, start=True, stop=False
                )
                nc.tensor.matmul(
                    opb[:], prs[0:1, t2_off : t2_off + cw], ones[0:1, 0:ncols],
                    start=False, stop=True,
                )
                nc.scalar.activation(bi[:, cc0 : cc0 + ncols], opb[:], Act.Copy)

            # r1 = sigmoid(fcw1/W . xsum + fcb1) -> [1, BL] in column order
            # [s0, s2, s1, s3]; each pair's chain (logit -> sigmoid -> scale/
            # bias outer products) only needs its own x tile.
            NPAIR = BL // 2
            r1 = small.tile([1, BL], f32)
            sc1 = small.tile([C1, BL], f32)
            bi1 = small.tile([C1, BL], f32)
            for half in range(2):
                lg1 = psmall.tile([1, NPAIR], f32, tag="sm", name=f"lg1{half}")
                nc.tensor.matmul(
                    lg1[:],
                    w1s[half * CIN : (half + 1) * CIN, 3 * C1 : 3 * C1 + 1].bitcast(f32),
                    m1[half * CIN : (half + 1) * CIN, :],
                    start=True, stop=True,
                )
                rr = r1[0:1, half * NPAIR : (half + 1) * NPAIR]
                nc.scalar.activation(
                    rr, lg1[:], Act.Sigmoid,
                    bias=prs[0:1, OFF_FCB1 : OFF_FCB1 + 1], scale=1.0,
                )
                outer_pair(OFF_S1, OFF_T11, OFF_T21, rr, NPAIR, sc1, bi1, half * NPAIR)

            def colmap(s):
                # column of sample s in r1 / sc1 / bi1 tiles
                return (s % 2) * NPAIR + s // 2

            # ---- block 1: conv(64->128); epilogue = one ScalarE op per tile:
            # Prelu(z*scale + bias, alpha) -> fp32r y1, with fused row-sum
            s1acc = small.tile([C1, BL * NT], f32)
            y1s = []
            for s in range(BL):
                y1 = y1pool.tile([C1, W + 2], f32r, tag="y1")
                # zero the two padding columns with an fp32r-writing DVE op
                # (memset cannot emit fp32r); inputs only feed a *0.0
                nc.vector.scalar_tensor_tensor(
                    y1[:, 0 : W + 2 : W + 1],
                    sc1[:, 0:2], 0.0, sc1[:, 0:2], Alu.mult, Alu.mult,
                )
                half = s % 2
                col = colmap(s)
                w1v = lambda k, h=half: w1s[
                    h * CIN : (h + 1) * CIN, k * C1 : (k + 1) * C1
                ]
                for t in range(NT):
                    zp = psum.tile([C1, WT], f32, tag="zp")
                    conv_taps(zp, w1v, xv(s), WT * t)
                    if t < 3:
                        # ScalarE drain: one fused Prelu(affine) + row-sum
                        nc.scalar.activation(
                            y1[:, 1 + WT * t : 1 + WT * (t + 1)], zp[:], Act.Prelu,
                            bias=bi1[:, col : col + 1], scale=sc1[:, col : col + 1],
                            alpha=SLOPE,
                            accum_out=s1acc[:, NT * s + t : NT * s + t + 1],
                        )
                    else:
                        # VectorE drain for 1 of 4 tiles: keeps ScalarE (which
                        # also runs the b1 drains at half rate out of PSUM)
                        # from becoming the block-1 bottleneck
                        ytmp = pmp.tile([C1, WT], f32, tag="ytmp")
                        nc.vector.tensor_scalar(
                            ytmp[:], zp[:],
                            sc1[:, col : col + 1], bi1[:, col : col + 1],
                            Alu.mult, Alu.add,
                        )
                        nc.vector.scalar_tensor_tensor(
                            y1[:, 1 + WT * t : 1 + WT * (t + 1)],
                            ytmp[:], SLOPE, ytmp[:], Alu.mult, Alu.max,
                            accum_out=s1acc[:, NT * s + t : NT * s + t + 1],
                        )
                y1s.append(y1)

            # r2 from block-1 output row sums (natural sample order)
            ta = small.tile([C1, BL], f32)
            tb = small.tile([C1, BL], f32)
            ssum = small.tile([C1, BL], f32)
            nc.vector.tensor_add(ta[:], s1acc[:, 0::NT], s1acc[:, 1::NT])
            nc.vector.tensor_add(tb[:], s1acc[:, 2::NT], s1acc[:, 3::NT])
            nc.vector.tensor_add(ssum[:], ta[:], tb[:])
            lg2 = psmall.tile([1, BL], f32, tag="sm")
            nc.tensor.matmul(
                lg2[:],
                w2s[:, 3 * C2 : 3 * C2 + 1].bitcast(f32),
                ssum[:], start=True, stop=True,
            )
            r2 = small.tile([1, BL], f32)
            nc.scalar.activation(
                r2[:], lg2[:], Act.Sigmoid,
                bias=prs[0:1, OFF_FCB2 : OFF_FCB2 + 1], scale=1.0,
            )
            sc2 = small.tile([C1, 2 * BL], f32)
            bi2 = small.tile([C1, 2 * BL], f32)
            outer_pair(OFF_S2, OFF_T12, OFF_T22, r2[:], BL, sc2, bi2, 0)
            outer_pair(OFF_S2 + C1, OFF_T12 + C1, OFF_T22 + C1, r2[:], BL, sc2, bi2, BL)

            # ---- block 2: conv(128->256); pool adjacent pairs straight from
            # PSUM (one-input 3D-AP max reduce), then Prelu(affine) at half
            # width.  Pool-before-affine is exact because scale>0.
            for s in range(BL):
                for c in range(2):
                    ot = outp.tile([C1, WO], f32, tag="ot")
                    w2v = lambda k, cc=c: w2s[:, k * C2 + C1 * cc : k * C2 + C1 * cc + C1]
                    for t in range(NT):
                        zp2 = psum.tile([C1, WT], f32, tag="zp")
                        conv_taps(zp2, w2v, y1s[s], WT * t)
                        pm = pmp.tile([C1, HT], f32, tag="pm")
                        nc.vector.tensor_reduce(
                            pm[:], zp2[:].rearrange("p (a b) -> p a b", b=2),
                            axis=Ax.X, op=Alu.max,
                        )
                        nc.scalar.activation(
                            ot[:, HT * t : HT * (t + 1)], pm[:], Act.Prelu,
                            bias=bi2[:, c * BL + s : c * BL + s + 1],
                            scale=sc2[:, c * BL + s : c * BL + s + 1],
                            alpha=SLOPE,
                        )
                        if t % 2 == 1:
                            nc.gpsimd.dma_start(
                                out=o_ap[s, C1 * c : C1 * (c + 1),
                                         HT * (t - 1) : HT * (t + 1)],
                                in_=ot[:, HT * (t - 1) : HT * (t + 1)],
                            )

    nc.compile()
    _built = nc
    return nc


def _pack_inputs(x, w1, b1, fcw1, fcb1, g1, be1, rm1, rv1,
                 w2, b2, fcw2, fcb2, g2, be2, rm2, rv2):
    f = np.float32
    s1 = (g1 / np.sqrt(rv1 + EPS)).astype(f)
    s2 = (g2 / np.sqrt(rv2 + EPS)).astype(f)
    prow = np.zeros(NPROW, f)
    prow[OFF_S1:OFF_S1 + C1] = s1
    prow[OFF_T11:OFF_T11 + C1] = b1[0] * s1
    prow[OFF_T21:OFF_T21 + C1] = be1 - rm1 * s1
    prow[OFF_S2:OFF_S2 + C2] = s2
    prow[OFF_T12:OFF_T12 + C2] = b2[0] * s2
    prow[OFF_T22:OFF_T22 + C2] = be2 - rm2 * s2
    prow[OFF_FCB1] = fcb1[0]
    prow[OFF_FCB2] = fcb2[0]
    prow[OFF_ONES:OFF_ONES + BL] = 1.0

    w1t = w1[0].transpose(1, 2, 0).reshape(CIN, 3 * C1).astype(f)
    w2t = w2[0].transpose(1, 2, 0).reshape(C1, 3 * C2).astype(f)
    wpk1 = np.zeros((2 * CIN, 3 * C1 + 1), f)
    wpk1[0:CIN, 0:3 * C1] = w1t
    wpk1[CIN:, 0:3 * C1] = w1t
    wpk1[0:CIN, 3 * C1] = fcw1[0] / W
    wpk1[CIN:, 3 * C1] = fcw1[0] / W
    wpk2 = np.zeros((C1, 3 * C2 + 1), f)
    wpk2[:, 0:3 * C2] = w2t
    wpk2[:, 3 * C2] = fcw2[0] / W

    com = {
        "wpk1": wpk1,
        "wpk2": wpk2,
        "prow": prow.reshape(1, NPROW),
    }
    xp = np.zeros((B, CIN, W + 2), f)
    xp[:, :, 1 : W + 1] = x
    return [
        {**com, "x": np.ascontiguousarray(xp[i * BL : (i + 1) * BL])}
        for i in range(N_CORES)
    ]


def _enable_trace():
    """Register the NTFF profile hook (absent antenv.axon_hooks on this image)
    and stub out the S3 artifact upload so trace=True works locally."""
    import sys
    import types

    import concourse.bass_utils as bu

    bu.upload_artifacts = lambda tmpdir: tmpdir
    if "antenv.axon_hooks" not in sys.modules:
        import antenv
        from trn_agent_boot.trn_boot import _ntff_profile_via_ctypes

        hooks = types.ModuleType("antenv.axon_hooks")
        _store = {"hook": _ntff_profile_via_ctypes("/opt/axon/libaxon_pjrt.so")}
        hooks.set_axon_ntff_profile_hook = lambda h: _store.__setitem__("hook", h)
        hooks.get_axon_ntff_profile_hook = lambda: _store["hook"]
        sys.modules["antenv.axon_hooks"] = hooks
        antenv.axon_hooks = hooks


def kernel(**inputs):
    global LAST_RESULT
    from concourse.bass_utils import run_bass_kernel_spmd

    if TRACE:
        _enable_trace()
    nc = _build()
    in_maps = _pack_inputs(**inputs)
    res = run_bass_kernel_spmd(nc, in_maps, list(range(N_CORES)), trace=TRACE)
    LAST_RESULT = res
    return np.concatenate([r["out"] for r in res.results], axis=0)
